# revision 19
# baseline (speedup 1.0000x reference)
"""Trainium2 Bass kernel for nn_PositionalEncoding_61151744360729.

out[b, s, n, :] = x[b, s, n, :] + ||x[b, s+1, n, :] - x[b, s, n, :]||_2
(with distance 0 at s = S-1).

Sharding: data-parallel on batch across 8 NeuronCores (64 batches/core).
On-core layout: partition p = b*2 + h (b = batch, h = sequence half),
free dim = frames*75 halfs, so every DMA is a large contiguous span per
partition. Each batch is padded host-side with a copy of its last frame,
which makes the last-frame distance exactly 0 with no special-casing.

All device I/O and compute is fp16 (tolerance is rel_err < 2e-2; fp16
end-to-end measures ~4e-4): halves HBM traffic vs f32 and enables the
DVE 2x perf mode on packed 2-byte ops.

The host uploads x transposed to [B, S, C, N] (c-major frames). With
coordinate planes contiguous inside each frame, the 3-coordinate sum
becomes two plane adds whose inner 25-element runs are packed (DVE 2x
eligible), and the distance broadcast in the final add has a packed
inner dim as well -- no replicate pass needed. Engine split measured on
HW: GpSimd tensor ops poison DVE's 2x mode (SBUF contention), so GpSimd
does nothing; ACT (which does not interfere) takes square and sqrt.
  DVE: shifted sub -> two plane adds -> broadcast final add  (all 2x)
  ACT: square, sqrt
  SP:  HWDGE DMA triggers
"""

import sys
from contextlib import ExitStack

for _p in ("/opt/trn_rl_repo", "/root/.axon_site/_ro/trn_rl_repo"):
    if _p not in sys.path:
        sys.path.insert(0, _p)

import numpy as np

import concourse.bass as bass
import concourse.tile as tile
from concourse import bacc, mybir
from concourse.bass_utils import run_bass_kernel_spmd

B, S, N, C = 512, 1024, 25, 3
FW = N * C                 # 75 values per frame (stored c-major: 3 planes of 25)
NCORES = 8
BC = B // NCORES           # 64 batches per core
H = 2                      # sequence halves -> 128 partitions
SH = S // H                # 512 frames per half
P = H * BC                 # 128 partitions
F = 64                     # frames per chunk per partition
K = SH // F                # 8 chunks
FN = F * N                 # 1600 node-distances per chunk
CH = F * FW                # 4800 values per chunk per partition
IN_FLAT = BC * (S + 1) * FW   # input padded by one copied frame per batch
OUT_FLAT = BC * S * FW

_cache = {}


def _build():
    f16 = mybir.dt.float16
    Af = mybir.ActivationFunctionType
    nc = bacc.Bacc(
        "TRN2", target_bir_lowering=False, debug=False, num_devices=NCORES
    )
    xin = nc.dram_tensor("xin", [IN_FLAT], f16, kind="ExternalInput")
    yout = nc.dram_tensor("yout", [OUT_FLAT], f16, kind="ExternalOutput")

    # chunk sizes in frames: small at the head (earlier compute start)
    # and at the tail (shorter drain chains)
    FS = [32, 32, 64, 64, 64, 64, 64, 32, 32, 32, 32]
    assert sum(FS) == SH
    S0 = [sum(FS[:i]) for i in range(len(FS))]   # start frame per chunk
    NK = len(FS)
    DVE_SQ = {NK - 4, NK - 3, NK - 2, NK - 1}    # tail: square on DVE

    with tile.TileContext(nc) as tc, ExitStack() as ctx:
        pin = ctx.enter_context(tc.tile_pool(name="pin", bufs=6))
        pmid = ctx.enter_context(tc.tile_pool(name="pmid", bufs=5))
        psm = ctx.enter_context(tc.tile_pool(name="psm", bufs=6))
        pout = ctx.enter_context(tc.tile_pool(name="pout", bufs=3))

        PF = 5  # input prefetch depth (chunks)
        D = 3   # sub+square lookahead depth (chunks)

        def issue_in(k):
            fk = FS[k]
            t = pin.tile([P, (fk + 1) * FW], f16)
            src = bass.AP(
                xin,
                S0[k] * FW,
                [[(S + 1) * FW, BC], [SH * FW, H], [1, (fk + 1) * FW]],
            )
            nc.sync.dma_start(t[:], src)
            return t

        in_tiles = [issue_in(k) for k in range(PF)]
        diff_tiles = {}

        def stage1(k):
            # sub + square for chunk k, issued D chunks ahead so the ACT
            # queue never head-blocks sqrt(k) behind square(k+1).
            fk = FS[k]
            in_t = in_tiles[k]
            diff_t = pmid.tile([P, fk * FW], f16)
            nc.vector.tensor_sub(
                diff_t[:], in_t[:, FW:(fk + 1) * FW], in_t[:, 0:fk * FW]
            )
            if k in DVE_SQ:
                nc.vector.tensor_mul(diff_t[:], diff_t[:], diff_t[:])
            else:
                nc.scalar.activation(diff_t[:], diff_t[:], Af.Square)
            diff_tiles[k] = diff_t

        for j in range(D):
            stage1(j)

        for k in range(NK):
            fk = FS[k]
            fn = fk * N
            if k + PF < NK:
                in_tiles.append(issue_in(k + PF))
            if k + D < NK:
                stage1(k + D)

            in_t = in_tiles[k]
            diff_t = diff_tiles.pop(k)

            # frames are c-major: coordinate planes at offsets 0/25/50
            sq4 = diff_t[:].rearrange("p (f c n) -> p f c n", f=fk, c=C, n=N)
            d2a_t = psm.tile([P, fn], f16)
            d2_t = psm.tile([P, fn], f16)
            d2a = d2a_t[:].rearrange("p (f n) -> p f n", f=fk, n=N)
            d2 = d2_t[:].rearrange("p (f n) -> p f n", f=fk, n=N)
            nc.vector.tensor_add(d2a, sq4[:, :, 0, :], sq4[:, :, 1, :])
            nc.vector.tensor_add(d2, d2a, sq4[:, :, 2, :])

            dist_t = psm.tile([P, fn], f16)
            nc.scalar.activation(dist_t[:], d2_t[:], Af.Sqrt)

            # broadcast add: dist operand [p, f, c(stride 0), n] keeps a
            # packed 25-wide inner run, so this stays in DVE 2x mode.
            out_t = pout.tile([P, fk * FW], f16)
            out4 = out_t[:].rearrange("p (f c n) -> p f c n", f=fk, c=C, n=N)
            in4 = in_t[:, 0:fk * FW].rearrange(
                "p (f c n) -> p f c n", f=fk, c=C, n=N
            )
            db = (
                dist_t[:]
                .rearrange("p (f n) -> p f n", f=fk, n=N)
                .unsqueeze(2)
                .broadcast_to([P, fk, C, N])
            )
            nc.vector.tensor_add(out4, in4, db)

            dst = bass.AP(
                yout,
                S0[k] * FW,
                [[S * FW, BC], [SH * FW, H], [1, fk * FW]],
            )
            nc.scalar.dma_start(dst, out_t[:])

    nc.compile()
    return nc


def kernel(x: np.ndarray, **_unused) -> np.ndarray:
    x = np.asarray(x)
    assert x.shape == (B, S, N, C), x.shape

    if "nc" not in _cache:
        _cache["nc"] = _build()
    nc = _cache["nc"]

    # device layout is [B, S, C, N] (c-major frames)
    xh = np.ascontiguousarray(
        x.transpose(0, 1, 3, 2), dtype=np.float16
    )
    in_maps = []
    for ci in range(NCORES):
        xc = xh[ci * BC:(ci + 1) * BC].reshape(BC, S * FW)
        xp = np.empty((BC, (S + 1) * FW), dtype=np.float16)
        xp[:, : S * FW] = xc
        # pad frame = copy of the last frame -> diff at s = S-1 is 0
        xp[:, S * FW:] = xc[:, (S - 1) * FW:]
        in_maps.append({"xin": xp.reshape(IN_FLAT)})

    res = run_bass_kernel_spmd(nc, in_maps, core_ids=list(range(NCORES)))
    _cache["last_results"] = res

    out = np.empty((B, S, N, C), dtype=np.float32)
    for ci in range(NCORES):
        yc = res.results[ci]["yout"].reshape(BC, S, C, N)
        out[ci * BC:(ci + 1) * BC] = yc.transpose(0, 1, 3, 2).astype(
            np.float32
        )
    return out


# revision 20
# speedup vs baseline: 1.0845x; 1.0845x over previous
"""Trainium2 Bass kernel for nn_PositionalEncoding_61151744360729.

out[b, s, n, :] = x[b, s, n, :] + ||x[b, s+1, n, :] - x[b, s, n, :]||_2
(with distance 0 at s = S-1).

Sharding: data-parallel on batch across 8 NeuronCores (64 batches/core).
On-core layout: partition p = b*2 + h (b = batch, h = sequence half),
free dim = frames*75 halfs, so every DMA is a large contiguous span per
partition. Each batch is padded host-side with a copy of its last frame,
which makes the last-frame distance exactly 0 with no special-casing.

All device I/O and compute is fp16 (tolerance is rel_err < 2e-2; fp16
end-to-end measures ~4e-4): halves HBM traffic vs f32 and enables the
DVE 2x perf mode on packed 2-byte ops.

The host uploads x transposed to [B, S, C, N] (c-major frames). With
coordinate planes contiguous inside each frame, the 3-coordinate sum
becomes two plane adds whose inner 25-element runs are packed (DVE 2x
eligible), and the distance broadcast in the final add has a packed
inner dim as well -- no replicate pass needed. Engine split measured on
HW: GpSimd tensor ops poison DVE's 2x mode (SBUF contention), so GpSimd
does nothing; ACT (which does not interfere) takes square and sqrt.
  DVE: shifted sub -> two plane adds -> broadcast final add  (all 2x)
  ACT: square, sqrt
  SP:  HWDGE DMA triggers
"""

import sys
from contextlib import ExitStack

for _p in ("/opt/trn_rl_repo", "/root/.axon_site/_ro/trn_rl_repo"):
    if _p not in sys.path:
        sys.path.insert(0, _p)

import numpy as np

import concourse.bass as bass
import concourse.tile as tile
from concourse import bacc, mybir
from concourse.bass_utils import run_bass_kernel_spmd

B, S, N, C = 512, 1024, 25, 3
FW = N * C                 # 75 values per frame (stored c-major: 3 planes of 25)
NCORES = 8
BC = B // NCORES           # 64 batches per core
H = 2                      # sequence halves -> 128 partitions
SH = S // H                # 512 frames per half
P = H * BC                 # 128 partitions
F = 64                     # frames per chunk per partition
K = SH // F                # 8 chunks
FN = F * N                 # 1600 node-distances per chunk
CH = F * FW                # 4800 values per chunk per partition
IN_FLAT = BC * (S + 1) * FW   # input padded by one copied frame per batch
OUT_FLAT = BC * S * FW

_cache = {}


def _build():
    f16 = mybir.dt.float16
    Af = mybir.ActivationFunctionType
    nc = bacc.Bacc(
        "TRN2", target_bir_lowering=False, debug=False, num_devices=NCORES
    )
    xin = nc.dram_tensor("xin", [IN_FLAT], f16, kind="ExternalInput")
    yout = nc.dram_tensor("yout", [OUT_FLAT], f16, kind="ExternalOutput")

    # chunk sizes in frames: small at the head (earlier compute start)
    # and at the tail (shorter drain chains)
    FS = [32, 32, 64, 64, 64, 64, 64, 32, 32, 32, 32]
    assert sum(FS) == SH
    S0 = [sum(FS[:i]) for i in range(len(FS))]   # start frame per chunk
    NK = len(FS)
    DVE_SQ = {NK - 4, NK - 3, NK - 2, NK - 1}    # tail: square on DVE

    with tile.TileContext(nc) as tc, ExitStack() as ctx:
        pin = ctx.enter_context(tc.tile_pool(name="pin", bufs=6))
        pmid = ctx.enter_context(tc.tile_pool(name="pmid", bufs=5))
        psm = ctx.enter_context(tc.tile_pool(name="psm", bufs=6))
        pout = ctx.enter_context(tc.tile_pool(name="pout", bufs=3))

        PF = 5  # input prefetch depth (chunks)
        D = 3   # sub+square lookahead depth (chunks)

        def issue_in(k):
            fk = FS[k]
            t = pin.tile([P, (fk + 1) * FW], f16)
            src = bass.AP(
                xin,
                S0[k] * FW,
                [[(S + 1) * FW, BC], [SH * FW, H], [1, (fk + 1) * FW]],
            )
            nc.sync.dma_start(t[:], src)
            return t

        in_tiles = [issue_in(k) for k in range(PF)]
        diff_tiles = {}

        def stage1(k):
            # sub + square for chunk k, issued D chunks ahead so the ACT
            # queue never head-blocks sqrt(k) behind square(k+1).
            fk = FS[k]
            in_t = in_tiles[k]
            diff_t = pmid.tile([P, fk * FW], f16)
            nc.vector.tensor_sub(
                diff_t[:], in_t[:, FW:(fk + 1) * FW], in_t[:, 0:fk * FW]
            )
            if k in DVE_SQ:
                nc.vector.tensor_mul(diff_t[:], diff_t[:], diff_t[:])
            else:
                nc.scalar.activation(diff_t[:], diff_t[:], Af.Square)
            diff_tiles[k] = diff_t

        for j in range(D):
            stage1(j)

        for k in range(NK):
            fk = FS[k]
            fn = fk * N
            if k + PF < NK:
                in_tiles.append(issue_in(k + PF))
            if k + D < NK:
                stage1(k + D)

            in_t = in_tiles[k]
            diff_t = diff_tiles.pop(k)

            # frames are c-major: coordinate planes at offsets 0/25/50
            sq4 = diff_t[:].rearrange("p (f c n) -> p f c n", f=fk, c=C, n=N)
            d2a_t = psm.tile([P, fn], f16)
            d2_t = psm.tile([P, fn], f16)
            d2a = d2a_t[:].rearrange("p (f n) -> p f n", f=fk, n=N)
            d2 = d2_t[:].rearrange("p (f n) -> p f n", f=fk, n=N)
            nc.vector.tensor_add(d2a, sq4[:, :, 0, :], sq4[:, :, 1, :])
            nc.vector.tensor_add(d2, d2a, sq4[:, :, 2, :])

            dist_t = psm.tile([P, fn], f16)
            nc.scalar.activation(dist_t[:], d2_t[:], Af.Sqrt)

            # broadcast add: dist operand [p, f, c(stride 0), n] keeps a
            # packed 25-wide inner run, so this stays in DVE 2x mode.
            out_t = pout.tile([P, fk * FW], f16)
            out4 = out_t[:].rearrange("p (f c n) -> p f c n", f=fk, c=C, n=N)
            in4 = in_t[:, 0:fk * FW].rearrange(
                "p (f c n) -> p f c n", f=fk, c=C, n=N
            )
            db = (
                dist_t[:]
                .rearrange("p (f n) -> p f n", f=fk, n=N)
                .unsqueeze(2)
                .broadcast_to([P, fk, C, N])
            )
            nc.vector.tensor_add(out4, in4, db)

            dst = bass.AP(
                yout,
                S0[k] * FW,
                [[S * FW, BC], [SH * FW, H], [1, fk * FW]],
            )
            nc.gpsimd.dma_start(dst, out_t[:])

    nc.compile()
    return nc


def kernel(x: np.ndarray, **_unused) -> np.ndarray:
    x = np.asarray(x)
    assert x.shape == (B, S, N, C), x.shape

    if "nc" not in _cache:
        _cache["nc"] = _build()
    nc = _cache["nc"]

    # device layout is [B, S, C, N] (c-major frames)
    xh = np.ascontiguousarray(
        x.transpose(0, 1, 3, 2), dtype=np.float16
    )
    in_maps = []
    for ci in range(NCORES):
        xc = xh[ci * BC:(ci + 1) * BC].reshape(BC, S * FW)
        xp = np.empty((BC, (S + 1) * FW), dtype=np.float16)
        xp[:, : S * FW] = xc
        # pad frame = copy of the last frame -> diff at s = S-1 is 0
        xp[:, S * FW:] = xc[:, (S - 1) * FW:]
        in_maps.append({"xin": xp.reshape(IN_FLAT)})

    res = run_bass_kernel_spmd(nc, in_maps, core_ids=list(range(NCORES)))
    _cache["last_results"] = res

    out = np.empty((B, S, N, C), dtype=np.float32)
    for ci in range(NCORES):
        yc = res.results[ci]["yout"].reshape(BC, S, C, N)
        out[ci * BC:(ci + 1) * BC] = yc.transpose(0, 1, 3, 2).astype(
            np.float32
        )
    return out


# revision 21
# speedup vs baseline: 1.1591x; 1.0688x over previous
"""Trainium2 Bass kernel for nn_PositionalEncoding_61151744360729.

out[b, s, n, :] = x[b, s, n, :] + ||x[b, s+1, n, :] - x[b, s, n, :]||_2
(with distance 0 at s = S-1).

Sharding: data-parallel on batch across 8 NeuronCores (64 batches/core).
On-core layout: partition p = b*2 + h (b = batch, h = sequence half),
free dim = frames*75 halfs, so every DMA is a large contiguous span per
partition. Each batch is padded host-side with a copy of its last frame,
which makes the last-frame distance exactly 0 with no special-casing.

All device I/O and compute is fp16 (tolerance is rel_err < 2e-2; fp16
end-to-end measures ~4e-4): halves HBM traffic vs f32 and enables the
DVE 2x perf mode on packed 2-byte ops.

The host uploads x transposed to [B, S, C, N] (c-major frames). With
coordinate planes contiguous inside each frame, the 3-coordinate sum
becomes two plane adds whose inner 25-element runs are packed (DVE 2x
eligible), and the distance broadcast in the final add has a packed
inner dim as well -- no replicate pass needed. Engine split measured on
HW: GpSimd tensor ops poison DVE's 2x mode (SBUF contention), so GpSimd
does nothing; ACT (which does not interfere) takes square and sqrt.
  DVE: shifted sub -> two plane adds -> broadcast final add  (all 2x)
  ACT: square, sqrt
  SP:  HWDGE DMA triggers
"""

import sys
from contextlib import ExitStack

for _p in ("/opt/trn_rl_repo", "/root/.axon_site/_ro/trn_rl_repo"):
    if _p not in sys.path:
        sys.path.insert(0, _p)

import numpy as np

import concourse.bass as bass
import concourse.tile as tile
from concourse import bacc, mybir
from concourse.bass_utils import run_bass_kernel_spmd

B, S, N, C = 512, 1024, 25, 3
FW = N * C                 # 75 values per frame (stored c-major: 3 planes of 25)
NCORES = 8
BC = B // NCORES           # 64 batches per core
H = 2                      # sequence halves -> 128 partitions
SH = S // H                # 512 frames per half
P = H * BC                 # 128 partitions
F = 64                     # frames per chunk per partition
K = SH // F                # 8 chunks
FN = F * N                 # 1600 node-distances per chunk
CH = F * FW                # 4800 values per chunk per partition
IN_FLAT = BC * (S + 1) * FW   # input padded by one copied frame per batch
OUT_FLAT = BC * S * FW

_cache = {}


def _build():
    f16 = mybir.dt.float16
    Af = mybir.ActivationFunctionType
    nc = bacc.Bacc(
        "TRN2", target_bir_lowering=False, debug=False, num_devices=NCORES
    )
    xin = nc.dram_tensor("xin", [IN_FLAT], f16, kind="ExternalInput")
    yout = nc.dram_tensor("yout", [OUT_FLAT], f16, kind="ExternalOutput")

    # chunk sizes in frames: small at the head (earlier compute start)
    # and at the tail (shorter drain chains)
    FS = [32, 32, 64, 64, 64, 64, 64, 32, 32, 32, 32]
    assert sum(FS) == SH
    S0 = [sum(FS[:i]) for i in range(len(FS))]   # start frame per chunk
    NK = len(FS)
    DVE_SQ = {NK - 4, NK - 3, NK - 2, NK - 1}    # tail: square on DVE

    with tile.TileContext(nc) as tc, ExitStack() as ctx:
        pin = ctx.enter_context(tc.tile_pool(name="pin", bufs=6))
        pmid = ctx.enter_context(tc.tile_pool(name="pmid", bufs=5))
        psm = ctx.enter_context(tc.tile_pool(name="psm", bufs=6))
        pout = ctx.enter_context(tc.tile_pool(name="pout", bufs=3))

        PF = 5  # input prefetch depth (chunks)
        D = 3   # sub+square lookahead depth (chunks)

        def issue_in(k):
            fk = FS[k]
            t = pin.tile([P, (fk + 1) * FW], f16)
            src = bass.AP(
                xin,
                S0[k] * FW,
                [[(S + 1) * FW, BC], [SH * FW, H], [1, (fk + 1) * FW]],
            )
            nc.sync.dma_start(t[:], src)
            return t

        in_tiles = [issue_in(k) for k in range(PF)]
        diff_tiles = {}

        def stage1(k):
            # sub + square for chunk k, issued D chunks ahead so the ACT
            # queue never head-blocks sqrt(k) behind square(k+1).
            fk = FS[k]
            in_t = in_tiles[k]
            diff_t = pmid.tile([P, fk * FW], f16)
            nc.vector.tensor_sub(
                diff_t[:], in_t[:, FW:(fk + 1) * FW], in_t[:, 0:fk * FW]
            )
            if k in DVE_SQ:
                nc.vector.tensor_mul(diff_t[:], diff_t[:], diff_t[:])
            else:
                nc.scalar.activation(diff_t[:], diff_t[:], Af.Square)
            diff_tiles[k] = diff_t

        for j in range(D):
            stage1(j)

        for k in range(NK):
            fk = FS[k]
            fn = fk * N
            if k + PF < NK:
                in_tiles.append(issue_in(k + PF))
            if k + D < NK:
                stage1(k + D)

            in_t = in_tiles[k]
            diff_t = diff_tiles.pop(k)

            # frames are c-major: coordinate planes at offsets 0/25/50
            sq4 = diff_t[:].rearrange("p (f c n) -> p f c n", f=fk, c=C, n=N)
            d2a_t = psm.tile([P, fn], f16)
            d2_t = psm.tile([P, fn], f16)
            d2a = d2a_t[:].rearrange("p (f n) -> p f n", f=fk, n=N)
            d2 = d2_t[:].rearrange("p (f n) -> p f n", f=fk, n=N)
            nc.vector.tensor_add(d2a, sq4[:, :, 0, :], sq4[:, :, 1, :])
            nc.vector.tensor_add(d2, d2a, sq4[:, :, 2, :])

            dist_t = psm.tile([P, fn], f16)
            nc.scalar.activation(dist_t[:], d2_t[:], Af.Sqrt)

            # broadcast add: dist operand [p, f, c(stride 0), n] keeps a
            # packed 25-wide inner run, so this stays in DVE 2x mode.
            out_t = pout.tile([P, fk * FW], f16)
            out4 = out_t[:].rearrange("p (f c n) -> p f c n", f=fk, c=C, n=N)
            in4 = in_t[:, 0:fk * FW].rearrange(
                "p (f c n) -> p f c n", f=fk, c=C, n=N
            )
            db = (
                dist_t[:]
                .rearrange("p (f n) -> p f n", f=fk, n=N)
                .unsqueeze(2)
                .broadcast_to([P, fk, C, N])
            )
            nc.vector.tensor_add(out4, in4, db)

            dst = bass.AP(
                yout,
                S0[k] * FW,
                [[S * FW, BC], [SH * FW, H], [1, fk * FW]],
            )
            nc.sync.dma_start(dst, out_t[:])

    nc.compile()
    return nc


def kernel(x: np.ndarray, **_unused) -> np.ndarray:
    x = np.asarray(x)
    assert x.shape == (B, S, N, C), x.shape

    if "nc" not in _cache:
        _cache["nc"] = _build()
    nc = _cache["nc"]

    # device layout is [B, S, C, N] (c-major frames)
    xh = np.ascontiguousarray(
        x.transpose(0, 1, 3, 2), dtype=np.float16
    )
    in_maps = []
    for ci in range(NCORES):
        xc = xh[ci * BC:(ci + 1) * BC].reshape(BC, S * FW)
        xp = np.empty((BC, (S + 1) * FW), dtype=np.float16)
        xp[:, : S * FW] = xc
        # pad frame = copy of the last frame -> diff at s = S-1 is 0
        xp[:, S * FW:] = xc[:, (S - 1) * FW:]
        in_maps.append({"xin": xp.reshape(IN_FLAT)})

    res = run_bass_kernel_spmd(nc, in_maps, core_ids=list(range(NCORES)))
    _cache["last_results"] = res

    out = np.empty((B, S, N, C), dtype=np.float32)
    for ci in range(NCORES):
        yc = res.results[ci]["yout"].reshape(BC, S, C, N)
        out[ci * BC:(ci + 1) * BC] = yc.transpose(0, 1, 3, 2).astype(
            np.float32
        )
    return out
